# revision 1
# baseline (speedup 1.0000x reference)
"""3-layer GAT (PyG GATConv, concat=False, mean-over-heads) on 8 TRN2
NeuronCores.

Strategy (graph/data parallel, per sharding hint):
  - Pad nodes to N_PAD; shard N_PAD/8 destination nodes per core.
  - Per layer: each core computes its shard of the fused feature table
    h_ext = x @ W_ext  (cols 0:256 = h, 256:260 = a_src, 260:264 = a_dst,
    pad to 320) on the TensorEngine, AllGathers the full table, then
    processes its destination-sorted edges: dma_gather of h_ext[src]
    rows, segment-softmax + weighted aggregation done as indicator-matrix
    matmuls accumulated in PSUM per 128-node destination block.
  - Segment softmax skips the max-subtraction (validated: |e| < 8 on all
    layers, exp is safe in f32).
  - Edges are grouped by (dst block, src half) because dma_gather indices
    are int16; each group is padded with dummy edges (dst_local=-1 so the
    indicator row is all-zero => zero contribution).
"""
import sys
sys.path.insert(0, "/opt/trn_rl_repo")
from dataclasses import dataclass

import numpy as np

import concourse.bass as bass
import concourse.mybir as mybir
from concourse.tile import TileContext
from concourse.bass_utils import run_bass_kernel_spmd
from concourse.library_config import mlp

F32 = mybir.dt.float32
BF16 = mybir.dt.bfloat16
I16 = mybir.dt.int16
AF = mybir.ActivationFunctionType
ALU = mybir.AluOpType
AX = mybir.AxisListType

C_IN, HC = 256, 256          # input feat, heads*hidden (4*64) for all layers
H, CH = 4, 64
NCORES = 8
P = 128
NEG = 0.2
R = 320                      # f32 compute row (256 h | 4 asrc | 4 adst | pad)
RT = 384                     # bf16 table row: 768B, %256B for dma_gather


@dataclass(frozen=True)
class Cfg:
    n: int            # real nodes
    n_pad: int        # padded nodes (multiple of 8*128)
    min_c: int        # minimum group capacity

    @property
    def shard(self):
        return self.n_pad // NCORES

    @property
    def nblk(self):
        return self.shard // P

    @property
    def half(self):
        return self.n_pad // 2

    @property
    def ng(self):
        return 2 * self.nblk


FULL = Cfg(n=50000, n_pad=50176, min_c=1280)


# ------------------------------------------------------------------ device --
def build_nc(C, cfg=FULL, nlayers=3):
    NSUB = C // P
    SHARD, NBLK, HALF, NG = cfg.shard, cfg.nblk, cfg.half, cfg.ng
    nc = bass.Bass(num_devices=NCORES)

    xT1_in = nc.dram_tensor("xT1", [2, P, SHARD], BF16, kind="ExternalInput")
    w1_in = nc.dram_tensor("w1", [P, 2, R], BF16, kind="ExternalInput")
    w2_in = nc.dram_tensor("w2", [CH, R], BF16, kind="ExternalInput")
    w3_in = nc.dram_tensor("w3", [CH, R], BF16, kind="ExternalInput")
    bias_in = nc.dram_tensor("bias", [3, P, CH], F32, kind="ExternalInput")
    iota_in = nc.dram_tensor("iota", [P, P], BF16, kind="ExternalInput")
    ident_in = nc.dram_tensor("ident", [P, P], F32, kind="ExternalInput")
    idx_in = nc.dram_tensor("idx", [NG, P, C // 16], I16, kind="ExternalInput")
    dstl_in = nc.dram_tensor("dstl", [NG, P, NSUB], BF16, kind="ExternalInput")
    out_ext = nc.dram_tensor("out", [SHARD, CH], F32, kind="ExternalOutput")

    h_shard = [nc.dram_tensor(f"hs{l}", [SHARD, RT], BF16, kind="Internal")
               for l in range(3)]
    h_full = [nc.dram_tensor(f"hf{l}", [cfg.n_pad, RT], BF16, kind="Internal",
                             addr_space="Shared") for l in range(3)]
    rg = [list(range(NCORES))]

    from contextlib import ExitStack
    with TileContext(nc) as tc:
        with ExitStack() as ctx:
            sbc = ctx.enter_context(tc.tile_pool(name="const", bufs=1))
            sb_xT = ctx.enter_context(tc.tile_pool(name="xT", bufs=2))
            sb_adst = ctx.enter_context(tc.tile_pool(name="adst", bufs=2))
            sb_lhs = ctx.enter_context(tc.tile_pool(name="lhs", bufs=4))
            sb_h = ctx.enter_context(tc.tile_pool(name="hd", bufs=3))
            sb_hg = ctx.enter_context(tc.tile_pool(name="hg", bufs=4))
            sb_idx = ctx.enter_context(tc.tile_pool(name="idx", bufs=4))
            sb_dstl = ctx.enter_context(tc.tile_pool(name="dstl", bufs=4))
            sb_ind = ctx.enter_context(tc.tile_pool(name="ind", bufs=4))
            sb_indT = ctx.enter_context(tc.tile_pool(name="indT", bufs=6))
            sb_sm = ctx.enter_context(tc.tile_pool(name="small", bufs=8))
            sb_out = ctx.enter_context(tc.tile_pool(name="outp", bufs=4))
            ps_h = ctx.enter_context(
                tc.tile_pool(name="ps_h", bufs=1, space="PSUM"))
            ps_agg = ctx.enter_context(
                tc.tile_pool(name="ps_agg", bufs=2, space="PSUM"))
            ps_tr = ctx.enter_context(
                tc.tile_pool(name="ps_tr", bufs=3, space="PSUM"))
            ps_sm = ctx.enter_context(
                tc.tile_pool(name="ps_sm", bufs=1, space="PSUM"))
            ps_tr2 = ctx.enter_context(
                tc.tile_pool(name="ps_tr2", bufs=1, space="PSUM"))
            nc.gpsimd.load_library(mlp)
            CH_G = 1024  # dma_gather hangs above ~1024 indices per call
            g_offs = [(o, min(CH_G, C - o)) for o in range(0, C, CH_G)]
            g_regs = {ni: nc.gpsimd.to_reg(ni)
                      for ni in sorted({ni for _, ni in g_offs})}
            iota = sbc.tile([P, P], BF16)
            nc.sync.dma_start(out=iota[:], in_=iota_in[:])
            ident = sbc.tile([P, P], F32)
            nc.sync.dma_start(out=ident[:], in_=ident_in[:])
            identb = sbc.tile([P, P], BF16)
            nc.vector.tensor_copy(out=identb[:], in_=ident[:])
            w1 = sbc.tile([P, 2, R], BF16)
            nc.sync.dma_start(out=w1[:], in_=w1_in[:])
            w2 = sbc.tile([CH, R], BF16)
            nc.sync.dma_start(out=w2[:], in_=w2_in[:])
            w3 = sbc.tile([CH, R], BF16)
            nc.sync.dma_start(out=w3[:], in_=w3_in[:])
            bias_t = [sbc.tile([P, CH], F32, tag=f"bias{l}", name=f"bias_t{l}")
                      for l in range(3)]
            for l in range(3):
                nc.sync.dma_start(out=bias_t[l][:], in_=bias_in[l])

            # layer-invariant edge data: load once, reuse all 3 layers
            idx_all = sbc.tile([P, NG, C // 16], I16)
            nc.sync.dma_start(
                out=idx_all[:],
                in_=idx_in[:].rearrange("g p c -> p g c"))
            dstl_all = sbc.tile([P, NG, NSUB], BF16)
            nc.sync.dma_start(
                out=dstl_all[:],
                in_=dstl_in[:].rearrange("g p s -> p g s"))

            xT_prev = None
            for l in range(nlayers):
                # ---------- dense phase: h_ext shard + a_src/a_dst ----------
                adst = sb_adst.tile([P, NBLK, 4], BF16)
                for m in range(NBLK):
                    ph = ps_h.tile([P, R], F32)
                    if l == 0:
                        for kc in range(2):
                            lt = sb_lhs.tile([P, P], BF16)
                            nc.sync.dma_start(
                                out=lt[:], in_=xT1_in[kc, :, m * P:(m + 1) * P])
                            nc.tensor.matmul(out=ph[:], lhsT=lt[:],
                                             rhs=w1[:, kc, :],
                                             start=(kc == 0), stop=(kc == 1))
                    else:
                        wl = w2 if l == 1 else w3
                        nc.tensor.matmul(out=ph[:],
                                         lhsT=xT_prev[:, m * P:(m + 1) * P],
                                         rhs=wl[:], start=True, stop=True)
                    ht = sb_h.tile([P, RT], BF16)
                    nc.vector.tensor_copy(out=ht[:, 0:R], in_=ph[:])
                    nc.vector.memset(ht[:, R:RT], 0.0)
                    nc.vector.tensor_copy(out=adst[:, m, :], in_=ht[:, 260:264])
                    nc.sync.dma_start(out=h_shard[l][m * P:(m + 1) * P, :],
                                      in_=ht[:])
                # ---------- all-gather the table ----------------------------
                nc.gpsimd.collective_compute(
                    "AllGather", ALU.bypass, replica_groups=rg,
                    ins=[h_shard[l][:]], outs=[h_full[l][:]])

                if l < 2:
                    xT_next = sb_xT.tile([CH, SHARD], BF16)

                # ---------- aggregation phase -------------------------------
                for b in range(NBLK):
                    pa = ps_agg.tile([P, 260], F32)
                    for hf in range(2):
                        g = 2 * b + hf
                        it = idx_all[:, g, :]
                        dt = dstl_all[:, g, :]
                        hg = sb_hg.tile([P, NSUB, RT], BF16)
                        for o, ni in g_offs:
                            nc.gpsimd.dma_gather(
                                hg[:, o // P:(o + ni) // P, :],
                                h_full[l][hf * HALF:(hf + 1) * HALF, :],
                                it[:, o // 16:(o + ni) // 16],
                                ni, g_regs[ni], RT)
                        # indicator for all subchunks in one op
                        ind = sb_ind.tile([P, NSUB, P], BF16)
                        nc.vector.tensor_tensor(
                            out=ind[:],
                            in0=dt.unsqueeze(2).broadcast_to([P, NSUB, P]),
                            in1=iota[:].unsqueeze(1).broadcast_to([P, NSUB, P]),
                            op=ALU.is_equal)
                        # a_dst expansion per subchunk: IndT @ adst_block
                        pad_ps = ps_sm.tile([P, NSUB * 4], F32)
                        for s in range(NSUB):
                            ptr = ps_tr.tile([P, P], BF16)
                            nc.tensor.transpose(ptr[:], ind[:, s, :], identb[:])
                            idT = sb_indT.tile([P, P], BF16)
                            nc.vector.tensor_copy(out=idT[:], in_=ptr[:])
                            nc.tensor.matmul(
                                out=pad_ps[:, s * 4:(s + 1) * 4], lhsT=idT[:],
                                rhs=adst[:, b, :], start=True, stop=True)
                        # e = lrelu(asrc + adst); exp(e) into cols 256:260
                        e1 = sb_sm.tile([P, NSUB, 4], F32, tag="e1")
                        nc.vector.tensor_tensor(
                            out=e1[:], in0=hg[:, :, 256:260],
                            in1=pad_ps[:].rearrange("p (s f) -> p s f", f=4),
                            op=ALU.add)
                        e2 = sb_sm.tile([P, NSUB, 4], F32, tag="e2")
                        nc.vector.tensor_scalar_mul(e2[:], e1[:], NEG)
                        nc.vector.tensor_tensor(out=e1[:], in0=e1[:],
                                                in1=e2[:], op=ALU.max)
                        nc.scalar.activation(hg[:, :, 256:260], e1[:], AF.Exp)
                        # msg *= exp (per head)
                        nc.vector.tensor_tensor(
                            out=hg[:, :, 0:256].rearrange(
                                "p s (h c) -> p s h c", c=CH),
                            in0=hg[:, :, 0:256].rearrange(
                                "p s (h c) -> p s h c", c=CH),
                            in1=hg[:, :, 256:260].unsqueeze(3).broadcast_to(
                                [P, NSUB, 4, CH]),
                            op=ALU.mult)
                        for s in range(NSUB):
                            nc.tensor.matmul(
                                out=pa[:], lhsT=ind[:, s, :],
                                rhs=hg[:, s, 0:260],
                                start=(hf == 0 and s == 0),
                                stop=(hf == 1 and s == NSUB - 1),
                                skip_group_check=True)
                    # ---------- block epilogue ------------------------------
                    den = sb_sm.tile([P, 4], F32, tag="den")
                    nc.vector.tensor_scalar_max(den[:], pa[:, 256:260], 1e-6)
                    rec = sb_sm.tile([P, 4], F32, tag="rec")
                    nc.vector.reciprocal(rec[:], den[:])
                    sc = sb_out.tile([P, HC], F32, tag="sc")
                    nc.vector.tensor_tensor(
                        out=sc[:].rearrange("p (h c) -> p h c", c=CH),
                        in0=pa[:, 0:256].rearrange("p (h c) -> p h c", c=CH),
                        in1=rec[:].unsqueeze(2).broadcast_to([P, 4, CH]),
                        op=ALU.mult)
                    red = sb_out.tile([P, CH], F32, tag="red")
                    nc.vector.tensor_reduce(
                        out=red[:],
                        in_=sc[:].rearrange("p (h c) -> p c h", c=CH),
                        axis=AX.X, op=ALU.add)
                    nc.vector.tensor_scalar_mul(red[:], red[:], 1.0 / H)
                    nc.vector.tensor_tensor(out=red[:], in0=red[:],
                                            in1=bias_t[l][:], op=ALU.add)
                    if l < 2:
                        nc.vector.tensor_scalar_max(red[:], red[:], 0.0)
                        if l == nlayers - 1:
                            nc.sync.dma_start(
                                out=out_ext[b * P:(b + 1) * P, :], in_=red[:])
                        else:
                            pt2 = ps_tr2.tile([CH, P], F32)
                            nc.tensor.transpose(pt2[:], red[:], ident[:])
                            nc.vector.tensor_copy(
                                out=xT_next[:, b * P:(b + 1) * P], in_=pt2[:])
                    else:
                        mx = sb_sm.tile([P, 1], F32, tag="mx")
                        nc.vector.tensor_reduce(out=mx[:], in_=red[:],
                                                axis=AX.X, op=ALU.max)
                        tt = sb_out.tile([P, CH], F32, tag="tt")
                        nc.vector.tensor_scalar(
                            out=tt[:], in0=red[:], scalar1=mx[:], scalar2=None,
                            op0=ALU.subtract)
                        ex = sb_out.tile([P, CH], F32, tag="ex")
                        ssum = sb_sm.tile([P, 1], F32, tag="ssum")
                        nc.scalar.activation(ex[:], tt[:], AF.Exp,
                                             accum_out=ssum[:])
                        ls = sb_sm.tile([P, 1], F32, tag="ls")
                        nc.scalar.activation(ls[:], ssum[:], AF.Ln)
                        nc.vector.tensor_scalar(
                            out=tt[:], in0=tt[:], scalar1=ls[:], scalar2=None,
                            op0=ALU.subtract)
                        nc.sync.dma_start(out=out_ext[b * P:(b + 1) * P, :],
                                          in_=tt[:])
                if l < 2:
                    xT_prev = xT_next

    return nc


# -------------------------------------------------------------------- host --
def prep(inputs, cfg=FULL):
    N, N_PAD, SHARD, HALF, NG = cfg.n, cfg.n_pad, cfg.shard, cfg.half, cfg.ng
    x = np.asarray(inputs["x"], np.float32)
    ei = np.asarray(inputs["edge_index"])
    src = np.concatenate([ei[0], np.arange(N, dtype=ei.dtype)]).astype(np.int64)
    dst = np.concatenate([ei[1], np.arange(N, dtype=ei.dtype)]).astype(np.int64)

    order = np.argsort(dst, kind="stable")
    src, dst = src[order], dst[order]
    blk = dst // P
    half = (src >= HALF).astype(np.int64)
    gid = blk * 2 + half
    order2 = np.argsort(gid, kind="stable")
    src, dst, gid = src[order2], dst[order2], gid[order2]

    ngt = (N_PAD // P) * 2
    gcnt = np.bincount(gid, minlength=ngt)
    C = max(cfg.min_c, int(np.ceil(gcnt.max() / P) * P))
    NSUB = C // P

    goff = np.zeros(ngt + 1, np.int64)
    np.cumsum(gcnt, out=goff[1:])
    pos = np.arange(len(src)) - goff[gid]

    idx_pad = np.zeros((ngt, C), np.int64)          # dummy src_local = 0
    dstl_pad = np.full((ngt, C), -1.0, np.float32)  # dummy dst_local = -1
    idx_pad[gid, pos] = src - (gid % 2) * HALF
    dstl_pad[gid, pos] = (dst % P).astype(np.float32)

    # wrap indices: idx_w[g, p, s] = idx_pad[g, s*16 + p%16]
    w = idx_pad.reshape(ngt, C // 16, 16).transpose(0, 2, 1)
    idx_w = np.tile(w, (1, 8, 1)).astype(np.int16)
    dstl_w = dstl_pad.reshape(ngt, NSUB, P).transpose(0, 2, 1).copy()

    x_pad = np.zeros((N_PAD, C_IN), np.float32)
    x_pad[:N] = x

    def wext(W, As, Ad):
        K = W.shape[0]
        We = np.zeros((K, R), np.float32)
        We[:, :HC] = W
        for hh in range(H):
            We[:, 256 + hh] = W[:, hh * CH:(hh + 1) * CH] @ As[hh]
            We[:, 260 + hh] = W[:, hh * CH:(hh + 1) * CH] @ Ad[hh]
        return We

    W1 = wext(np.asarray(inputs["W1"], np.float32),
              np.asarray(inputs["as1"], np.float32),
              np.asarray(inputs["ad1"], np.float32)).reshape(2, P, R)
    W1 = np.ascontiguousarray(W1.transpose(1, 0, 2))  # [P, 2, R]
    W2 = wext(np.asarray(inputs["W2"], np.float32),
              np.asarray(inputs["as2"], np.float32),
              np.asarray(inputs["ad2"], np.float32))
    W3 = wext(np.asarray(inputs["W3"], np.float32),
              np.asarray(inputs["as3"], np.float32),
              np.asarray(inputs["ad3"], np.float32))
    bias = np.stack([
        np.tile(np.asarray(inputs[f"b{i}"], np.float32)[None, :], (P, 1))
        for i in (1, 2, 3)])
    iota = np.tile(np.arange(P, dtype=np.float32)[None, :], (P, 1))
    ident = np.eye(P, dtype=np.float32)

    import ml_dtypes
    bf = ml_dtypes.bfloat16
    W1b, W2b, W3b = W1.astype(bf), W2.astype(bf), W3.astype(bf)
    in_maps = []
    for c in range(NCORES):
        xs = x_pad[c * SHARD:(c + 1) * SHARD].T  # [C_IN, SHARD]
        in_maps.append({
            "xT1": np.ascontiguousarray(xs.reshape(2, P, SHARD)).astype(bf),
            "w1": W1b, "w2": W2b, "w3": W3b, "bias": bias,
            "iota": iota.astype(bf), "ident": ident,
            "idx": np.ascontiguousarray(idx_w[c * NG:(c + 1) * NG]),
            "dstl": np.ascontiguousarray(
                dstl_w[c * NG:(c + 1) * NG]).astype(bf),
        })
    return C, in_maps


def split_sync_waits(nc, max_waits=1):
    """This container's walrus accepts at most one sync-wait per
    instruction; hoist extras onto injected same-engine InstNoOps."""
    n_new = 0
    for f in nc.m.functions:
        for bb in f.blocks:
            new_insts = []
            for inst in bb.instructions:
                si = inst.sync_info
                waits = list(si.on_wait) if si is not None and si.on_wait else []
                if len(waits) > max_waits:
                    for w in waits[:-max_waits]:
                        nop = mybir.InstNoOp(
                            name=f"{inst.name}-hw{n_new}", ins=[], outs=[])
                        nop.engine = inst.engine
                        nop.sync_info = mybir.SyncInfo(on_wait=[w], on_update=[])
                        new_insts.append(nop)
                        n_new += 1
                    si.on_wait = waits[-max_waits:]
                new_insts.append(inst)
            bb.instructions = new_insts
    return n_new


_nc_cache = {}


def kernel(trace=False, **inputs):
    C, in_maps = prep(inputs, FULL)
    if C not in _nc_cache:
        nc = build_nc(C, FULL)
        from concourse.library_overlay import lower_extended_insts
        lower_extended_insts(nc)
        split_sync_waits(nc)
        _nc_cache[C] = nc
    nc = _nc_cache[C]
    import time as _time
    res = None
    for attempt in range(3):
        try:
            res = run_bass_kernel_spmd(nc, in_maps,
                                       core_ids=list(range(NCORES)),
                                       trace=trace)
            break
        except Exception:
            # transient device-unrecoverable states clear after the axon
            # worker restarts; retry
            if attempt == 2:
                raise
            _time.sleep(20)
    out = np.concatenate([res.results[c]["out"] for c in range(NCORES)], axis=0)
    kernel.last_result = res
    return out[:FULL.n].astype(np.float32)

